# revision 2
# baseline (speedup 1.0000x reference)
"""Trainium2 Bass kernel for causal self-attention + out-proj + residual + LayerNorm.

Sharding: heads (tensor-parallel) across 8 cores for QKV+attention (kernel A),
then sequence-parallel across 8 cores for out-proj + residual + LN (kernel B).

Kernel A (per core, 2 heads):
 - x.T is pre-transposed and bf16-cast on the host (no PE transposes).
 - QKV projection in bf16 (FWL weight loads), V transposed to natural layout.
 - Scores: per-head bf16 matmuls row-tiled to array halves (concurrent).
 - exp split across ACT (true exp -> bf16) and DVE (Schraudolph int16 bit
   trick: bf16(exp(x)) ~ bitcast(int16(x*128*log2e + 16250))), masking and
   denominator-reciprocal broadcast on GpSimd.
 - PV in bf16 with a fused ones-column (M=65) computing the softmax
   denominator in the same matmul.
Kernel B: out-proj in bf16 + residual + LayerNorm (fp32 stats).
"""

import math
from contextlib import ExitStack

import numpy as np
import ml_dtypes

import concourse.bass as bass
import concourse.tile as tile
from concourse import bacc, mybir
from concourse.bass_utils import run_bass_kernel_spmd

# NTFF-trace shim: make run_bass_kernel_spmd(trace=True) usable in containers
# whose antenv lacks axon_hooks (harmless when tracing is off).
def _install_trace_shim():
    import sys, types
    try:
        import antenv.axon_hooks  # noqa: F401
        return
    except ImportError:
        pass
    try:
        import antenv
        from trn_agent_boot.trn_boot import _ntff_profile_via_ctypes
        hook = _ntff_profile_via_ctypes("/opt/axon/libaxon_pjrt.so")
        mod = types.ModuleType("antenv.axon_hooks")
        mod.get_axon_ntff_profile_hook = lambda: hook
        mod.set_axon_ntff_profile_hook = lambda h: None
        sys.modules["antenv.axon_hooks"] = mod
        antenv.axon_hooks = mod
        import concourse.bass_utils as _bu
        _bu.upload_artifacts = lambda tmpdir: "local://skipped"
    except Exception:
        pass


_install_trace_shim()

F32 = mybir.dt.float32
BF16 = mybir.dt.bfloat16
I16 = mybir.dt.int16
EXP = mybir.ActivationFunctionType.Exp
SQRT = mybir.ActivationFunctionType.Sqrt
BF = ml_dtypes.bfloat16

T_FULL = 4096
D = 1024
HEADS = 16
NCORES = 8
LN_EPS = 1e-5

# Schraudolph bf16 exp constants (validated on HW: DVE rounds to nearest)
LOG2E = 1.4426950408889634
SCH_A = 128.0 * LOG2E
SCH_B = 127.0 * 128.0 - 6.0

# exp column split: ACT does [0:ACOLS), DVE does [ACOLS:1024) of each step
ACOLS = 480

_CACHE = {}
LAST_RESULTS = {}


def build_kernel_a(T=T_FULL):
    """Per core: 2 heads. Computes A.T = softmax(QK^T/sqrt(d)) @ V, transposed
    ([128 = 2*64 head dims, T] bf16) and normalized."""
    nc = bacc.Bacc("TRN2", target_bir_lowering=False, debug=False)
    KD = D // 128          # 8 contraction tiles over D
    NT = T // 128          # token tiles of 128
    NQ = T // 512          # query chunks of 512

    xt_d = nc.dram_tensor("xt", [128, KD, T], BF16, kind="ExternalInput")
    id_d = nc.dram_tensor("ident", [128, 128], BF16, kind="ExternalInput")
    tm_d = nc.dram_tensor("trimask", [128, 128], BF16, kind="ExternalInput")
    wq_d = nc.dram_tensor("wq_t", [128, KD, 128], BF16, kind="ExternalInput")
    wk_d = nc.dram_tensor("wk_t", [128, KD, 128], BF16, kind="ExternalInput")
    wv_d = nc.dram_tensor("wv_t", [128, KD, 128], BF16, kind="ExternalInput")
    bq_d = nc.dram_tensor("bq", [128, 1], F32, kind="ExternalInput")
    bk_d = nc.dram_tensor("bk", [128, 1], F32, kind="ExternalInput")
    bv_d = nc.dram_tensor("bv", [128, 1], F32, kind="ExternalInput")
    at_d = nc.dram_tensor("at_out", [128, T], BF16, kind="ExternalOutput")

    with tile.TileContext(nc) as tc, ExitStack() as ctx:
        const = ctx.enter_context(tc.tile_pool(name="const", bufs=1))
        persist = ctx.enter_context(tc.tile_pool(name="persist", bufs=1))

        ident = const.tile([128, 128], BF16)
        nc.sync.dma_start(ident[:], id_d.ap())
        trimask = const.tile([128, 128], BF16)
        nc.sync.dma_start(trimask[:], tm_d.ap())
        wq_sb = const.tile([128, KD, 128], BF16, tag="wq")
        wk_sb = const.tile([128, KD, 128], BF16, tag="wk")
        wv_sb = const.tile([128, KD, 128], BF16, tag="wv")
        nc.sync.dma_start(wq_sb[:], wq_d.ap())
        nc.sync.dma_start(wk_sb[:], wk_d.ap())
        nc.sync.dma_start(wv_sb[:], wv_d.ap())
        bq_sb = const.tile([128, 1], F32, tag="bq")
        bk_sb = const.tile([128, 1], F32, tag="bk")
        bv_sb = const.tile([128, 1], F32, tag="bv")
        nc.sync.dma_start(bq_sb[:], bq_d.ap())
        nc.sync.dma_start(bk_sb[:], bk_d.ap())
        nc.sync.dma_start(bv_sb[:], bv_d.ap())

        xt_sb = persist.tile([128, KD, T], BF16, tag="xt")
        nc.sync.dma_start(xt_sb[:], xt_d.ap())

        # V natural layout [k-token part, kt, 130]: per head 64 V cols + ones
        v_sb = persist.tile([128, NT, 130], BF16, tag="v")
        nc.gpsimd.memset(v_sb[:, :, 64:65], 1.0)
        nc.gpsimd.memset(v_sb[:, :, 129:130], 1.0)
        qt_sb = persist.tile([128, T], BF16, tag="qt")
        kt_sb = persist.tile([128, T], BF16, tag="kt")
        at_sb = persist.tile([128, T], BF16, tag="at")

        # ---- Phase 1: QKV projection per 512-token chunk ----
        with ExitStack() as ctx2:
            vtp = ctx2.enter_context(tc.tile_pool(name="vtp", bufs=2))
            mm_ps = ctx2.enter_context(tc.tile_pool(name="mm_ps", bufs=2, space="PSUM"))
            tr_ps = ctx2.enter_context(tc.tile_pool(name="tr_ps", bufs=2, space="PSUM"))

            for vc in range(NQ):
                c_sl = slice(vc * 512, (vc + 1) * 512)
                # Q and K
                for nm, w_sb, b_sb, o_sb in (("q", wq_sb, bq_sb, qt_sb),
                                             ("k", wk_sb, bk_sb, kt_sb)):
                    pps = mm_ps.tile([128, 512], F32, tag="mm", name=f"pps_{nm}_{vc}")
                    for kt in range(KD):
                        nc.tensor.matmul(pps[:], w_sb[:, kt, :], xt_sb[:, kt, c_sl],
                                         start=(kt == 0), stop=(kt == KD - 1))
                    nc.vector.tensor_scalar(out=o_sb[:, c_sl], in0=pps[:],
                                            scalar1=b_sb[:], scalar2=None,
                                            op0=mybir.AluOpType.add)
                # V -> natural layout via PE transpose
                vps = mm_ps.tile([128, 512], F32, tag="mm", name=f"vps_{vc}")
                for kt in range(KD):
                    nc.tensor.matmul(vps[:], wv_sb[:, kt, :], xt_sb[:, kt, c_sl],
                                     start=(kt == 0), stop=(kt == KD - 1))
                vt_c = vtp.tile([128, 512], BF16, tag="vt", name=f"vt_{vc}")
                nc.vector.tensor_scalar(out=vt_c[:], in0=vps[:], scalar1=bv_sb[:],
                                        scalar2=None, op0=mybir.AluOpType.add)
                tpv = tr_ps.tile([128, 4, 128], BF16, tag="tr", name=f"tpv_{vc}")
                for q in range(4):
                    nc.tensor.transpose(tpv[:, q, :], vt_c[:, q * 128:(q + 1) * 128],
                                        ident[:])
                nc.vector.tensor_copy(v_sb[:, vc * 4:(vc + 1) * 4, 0:64],
                                      tpv[:, :, 0:64])
                nc.vector.tensor_copy(v_sb[:, vc * 4:(vc + 1) * 4, 65:129],
                                      tpv[:, :, 64:128])

        # ---- Phase 2: attention ----
        # Per q-chunk of 512, per k-tile of 128: both heads' scores run
        # concurrently in the two array halves (row tiling via base_partition),
        # exp is split ACT/DVE, PV lags one step (software pipeline).
        with ExitStack() as ctx3:
            e_pool = ctx3.enter_context(tc.tile_pool(name="e_pool", bufs=4))
            rb_pool = ctx3.enter_context(tc.tile_pool(name="rb_pool", bufs=2))
            s_ps = ctx3.enter_context(tc.tile_pool(name="s_ps", bufs=2, space="PSUM"))
            pv_ps = ctx3.enter_context(tc.tile_pool(name="pv_ps", bufs=2, space="PSUM"))

            for qc in range(NQ):
                nkt = 4 * (qc + 1)
                q_sl = slice(qc * 512, (qc + 1) * 512)
                pv = [pv_ps.tile([65, 512], F32, tag=f"pv{h}", name=f"pv{h}_{qc}")
                      for h in (0, 1)]

                def emit_pv(kt, esb):
                    for h in (0, 1):
                        nc.tensor.matmul(pv[h][:, :],
                                         v_sb[:, kt, 65 * h:65 * h + 65],
                                         esb[:, 512 * h:512 * h + 512],
                                         start=(kt == 0), stop=(kt == nkt - 1),
                                         skip_group_check=True)

                prev = None
                for kt in range(nkt):
                    sp = s_ps.tile([128, 1024], F32, tag="s", name=f"s_{qc}_{kt}")
                    for h in (0, 1):
                        h_sl = slice(64 * h, 64 * h + 64)
                        nc.tensor.matmul(sp[:, 512 * h:512 * h + 512],
                                         kt_sb[h_sl, kt * 128:(kt + 1) * 128],
                                         qt_sb[h_sl, q_sl],
                                         start=True, stop=True)
                    esb = e_pool.tile([128, 1024], BF16, tag="e", name=f"e_{qc}_{kt}")
                    nc.scalar.activation(out=esb[:, 0:ACOLS], in_=sp[:, 0:ACOLS],
                                         func=EXP)
                    nc.vector.tensor_scalar(out=esb[:, ACOLS:1024].bitcast(I16),
                                            in0=sp[:, ACOLS:1024],
                                            scalar1=SCH_A, scalar2=SCH_B,
                                            op0=mybir.AluOpType.mult,
                                            op1=mybir.AluOpType.add)
                    if kt >= nkt - 4:
                        o = kt * 128 - qc * 512
                        for h in (0, 1):
                            if o > 0:
                                nc.gpsimd.memset(esb[:, 512 * h:512 * h + o], 0.0)
                            nc.gpsimd.tensor_mul(esb[:, 512 * h + o:512 * h + o + 128],
                                                 esb[:, 512 * h + o:512 * h + o + 128],
                                                 trimask[:])
                    if prev is not None:
                        emit_pv(kt - 1, prev)
                    prev = esb
                emit_pv(nkt - 1, prev)

                for h in (0, 1):
                    r1 = rb_pool.tile([1, 512], F32, tag="r1", name=f"r1{h}_{qc}")
                    nc.vector.reciprocal(r1[:], pv[h][64:65, :])
                    rb = rb_pool.tile([128, 512], F32, tag="rb", name=f"rb{h}_{qc}")
                    nc.gpsimd.partition_broadcast(rb[:], r1[:], channels=128)
                    nc.vector.tensor_mul(at_sb[64 * h:64 * h + 64, q_sl],
                                         pv[h][0:64, :], rb[64 * h:64 * h + 64, :])
                nc.sync.dma_start(at_d.ap()[:, q_sl], at_sb[:, q_sl])

    nc.compile()
    return nc


def build_kernel_b(T=T_FULL):
    """Per core: rows slice of T/8 tokens: out-proj (bf16) + residual (+bout
    folded on host into xb) + LayerNorm*gamma+beta."""
    nc = bacc.Bacc("TRN2", target_bir_lowering=False, debug=False)
    Tc = T // NCORES
    KD = D // 128

    at_d = nc.dram_tensor("at", [128, KD, Tc], BF16, kind="ExternalInput")
    wo_d = nc.dram_tensor("wout_t", [128, KD, D], BF16, kind="ExternalInput")
    xb_d = nc.dram_tensor("xb", [Tc, D], F32, kind="ExternalInput")
    g_d = nc.dram_tensor("gamma", [1, D], F32, kind="ExternalInput")
    be_d = nc.dram_tensor("beta", [1, D], F32, kind="ExternalInput")
    y_d = nc.dram_tensor("y", [Tc, D], F32, kind="ExternalOutput")

    with tile.TileContext(nc) as tc, ExitStack() as ctx:
        const = ctx.enter_context(tc.tile_pool(name="const", bufs=1))
        work = ctx.enter_context(tc.tile_pool(name="work", bufs=2))
        stats = ctx.enter_context(tc.tile_pool(name="stats", bufs=4))
        ps = ctx.enter_context(tc.tile_pool(name="ps", bufs=4, space="PSUM"))

        at_sb = const.tile([128, KD, Tc], BF16, tag="at")
        nc.sync.dma_start(at_sb[:], at_d.ap())
        wo_sb = const.tile([128, KD, D], BF16, tag="wo")
        nc.sync.dma_start(wo_sb[:], wo_d.ap())
        gam_b = const.tile([128, D], F32, tag="gam")
        bet_b = const.tile([128, D], F32, tag="bet")
        nc.gpsimd.dma_start(gam_b[:], g_d.ap().to_broadcast([128, D]))
        nc.gpsimd.dma_start(bet_b[:], be_d.ap().to_broadcast([128, D]))
        eps_sb = const.tile([128, 1], F32, tag="eps")
        nc.vector.memset(eps_sb[:], LN_EPS)

        for tt in range(Tc // 128):
            t_sl = slice(tt * 128, (tt + 1) * 128)
            xb_t = work.tile([128, D], F32, tag="xb")
            nc.sync.dma_start(xb_t[:], xb_d.ap()[t_sl, :])
            y_t = work.tile([128, D], F32, tag="y")
            for j in (0, 1):
                pp = ps.tile([128, 512], F32, tag="pp")
                for kt in range(KD):
                    nc.tensor.matmul(pp[:], at_sb[:, kt, t_sl],
                                     wo_sb[:, kt, j * 512:(j + 1) * 512],
                                     start=(kt == 0), stop=(kt == KD - 1))
                nc.vector.tensor_add(y_t[:, j * 512:(j + 1) * 512], pp[:],
                                     xb_t[:, j * 512:(j + 1) * 512])
            st = stats.tile([128, 2, 6], F32, tag="st")
            nc.vector.bn_stats(st[:, 0, :], y_t[:, 0:512])
            nc.vector.bn_stats(st[:, 1, :], y_t[:, 512:1024])
            mv = stats.tile([128, 2], F32, tag="mv")
            nc.vector.bn_aggr(mv[:], st[:])
            sq = stats.tile([128, 1], F32, tag="sq")
            nc.scalar.activation(out=sq[:], in_=mv[:, 1:2], func=SQRT,
                                 bias=eps_sb[:], scale=1.0)
            rstd = stats.tile([128, 1], F32, tag="rstd")
            nc.vector.reciprocal(rstd[:], sq[:])
            nc.vector.tensor_scalar(out=y_t[:], in0=y_t[:], scalar1=mv[:, 0:1],
                                    scalar2=rstd[:], op0=mybir.AluOpType.subtract,
                                    op1=mybir.AluOpType.mult)
            nc.vector.tensor_mul(y_t[:], y_t[:], gam_b[:])
            nc.vector.tensor_add(y_t[:], y_t[:], bet_b[:])
            nc.sync.dma_start(y_d.ap()[t_sl, :], y_t[:])

    nc.compile()
    return nc


def _get_kernels(T=T_FULL):
    if T not in _CACHE:
        _CACHE[T] = (build_kernel_a(T), build_kernel_b(T))
    return _CACHE[T]


def _tile_kd(a):
    """[D, M] -> [128, D//128, M] with row = kt*128 + p."""
    Dd, M = a.shape
    return np.ascontiguousarray(a.reshape(Dd // 128, 128, M).transpose(1, 0, 2))


def kernel(x, Wqkv, bqkv, Wout, bout, gamma, beta):
    x = np.asarray(x, dtype=np.float32)
    Wqkv = np.asarray(Wqkv, dtype=np.float32)
    bqkv = np.asarray(bqkv, dtype=np.float32)
    Wout = np.asarray(Wout, dtype=np.float32)
    bout = np.asarray(bout, dtype=np.float32)
    gamma = np.asarray(gamma, dtype=np.float32)
    beta = np.asarray(beta, dtype=np.float32)

    B, T, D_ = x.shape
    assert B == 1 and D_ == D
    d = D // HEADS
    scale = d ** -0.5
    x2d = np.ascontiguousarray(x[0])
    ident = np.eye(128, dtype=np.float32).astype(BF)
    tri = np.triu(np.ones((128, 128), np.float32)).astype(BF)

    nc_a, nc_b = _get_kernels(T)

    xt = _tile_kd(x2d.T.astype(BF))            # [128, 8, T] bf16
    in_maps_a = []
    for c in range(NCORES):
        r = slice(c * 128, (c + 1) * 128)
        wq = Wqkv[0 * D:1 * D][r]
        wk = Wqkv[1 * D:2 * D][r] * scale
        wv = Wqkv[2 * D:3 * D][r]
        in_maps_a.append({
            "xt": xt,
            "ident": ident,
            "trimask": tri,
            "wq_t": _tile_kd(wq.T.astype(BF)),
            "wk_t": _tile_kd(wk.T.astype(BF)),
            "wv_t": _tile_kd(wv.T.astype(BF)),
            "bq": np.ascontiguousarray(bqkv[0 * D:1 * D][r].reshape(128, 1)),
            "bk": np.ascontiguousarray((bqkv[1 * D:2 * D][r] * scale).reshape(128, 1)),
            "bv": np.ascontiguousarray(bqkv[2 * D:3 * D][r].reshape(128, 1)),
        })
    res_a = run_bass_kernel_spmd(nc_a, in_maps_a, core_ids=list(range(NCORES)))
    LAST_RESULTS["a"] = res_a
    at_full = np.concatenate([np.asarray(res_a.results[c]["at_out"])
                              for c in range(NCORES)], axis=0)  # [D, T] bf16

    Tc = T // NCORES
    wo_tiled = _tile_kd(Wout.T.astype(BF))     # [128, 8, D] bf16
    in_maps_b = []
    for c in range(NCORES):
        t_sl = slice(c * Tc, (c + 1) * Tc)
        at_c = at_full[:, t_sl]                # [D, Tc] bf16
        in_maps_b.append({
            "at": _tile_kd(at_c),
            "wout_t": wo_tiled,
            "xb": np.ascontiguousarray(x2d[t_sl] + bout[None, :]),
            "gamma": np.ascontiguousarray(gamma.reshape(1, D)),
            "beta": np.ascontiguousarray(beta.reshape(1, D)),
        })
    res_b = run_bass_kernel_spmd(nc_b, in_maps_b, core_ids=list(range(NCORES)))
    LAST_RESULTS["b"] = res_b
    y = np.concatenate([res_b.results[c]["y"] for c in range(NCORES)], axis=0)
    return y.reshape(1, T, D).astype(np.float32)


# revision 3
# speedup vs baseline: 1.2471x; 1.2471x over previous
"""Trainium2 Bass kernel for causal self-attention + out-proj + residual + LayerNorm.

Sharding: heads (tensor-parallel) across 8 cores for QKV+attention (kernel A),
then sequence-parallel across 8 cores for out-proj + residual + LN (kernel B).

Kernel A (per core, 2 heads), fused per 512-token q-chunk:
 - x.T is pre-transposed and bf16-cast on the host (no PE transposes for x).
 - QKV projection for chunk qc in bf16, biases added on ACT, V transposed to
   natural token-major layout; attention for chunk qc follows immediately
   (causality means it only needs chunks <= qc), overlapping proj and
   attention across engines.
 - Scores: per-head bf16 matmuls row-tiled to the two array halves.
 - exp alternates whole steps between ACT (true exp -> bf16) and DVE
   (Schraudolph int16 bit trick: bitcast(int16(x*128*log2e + 16250))).
 - Masking and reciprocal-broadcast on GpSimd; PV in bf16 with a fused
   ones-column (M=65) producing the softmax denominator in the same matmul.
Kernel B: out-proj in bf16 + residual + LayerNorm (fp32 stats), no gpsimd
(avoids its expensive software-DGE drain at teardown).
"""

import math
from contextlib import ExitStack

import numpy as np
import ml_dtypes

import concourse.bass as bass
import concourse.tile as tile
from concourse import bacc, mybir
from concourse.bass_utils import run_bass_kernel_spmd

# NTFF-trace shim: make run_bass_kernel_spmd(trace=True) usable in containers
# whose antenv lacks axon_hooks (harmless when tracing is off).
def _install_trace_shim():
    import sys, types
    try:
        import antenv.axon_hooks  # noqa: F401
        return
    except ImportError:
        pass
    try:
        import antenv
        from trn_agent_boot.trn_boot import _ntff_profile_via_ctypes
        hook = _ntff_profile_via_ctypes("/opt/axon/libaxon_pjrt.so")
        mod = types.ModuleType("antenv.axon_hooks")
        mod.get_axon_ntff_profile_hook = lambda: hook
        mod.set_axon_ntff_profile_hook = lambda h: None
        sys.modules["antenv.axon_hooks"] = mod
        antenv.axon_hooks = mod
        import concourse.bass_utils as _bu
        _bu.upload_artifacts = lambda tmpdir: "local://skipped"
    except Exception:
        pass


_install_trace_shim()

F32 = mybir.dt.float32
BF16 = mybir.dt.bfloat16
I16 = mybir.dt.int16
EXP = mybir.ActivationFunctionType.Exp
IDENT_FN = mybir.ActivationFunctionType.Identity
SQRT = mybir.ActivationFunctionType.Sqrt
BF = ml_dtypes.bfloat16

T_FULL = 4096
D = 1024
HEADS = 16
NCORES = 8
LN_EPS = 1e-5

# Schraudolph bf16 exp constants (validated on HW: DVE rounds to nearest)
LOG2E = 1.4426950408889634
SCH_A = 128.0 * LOG2E
SCH_B = 127.0 * 128.0 - 6.0

_CACHE = {}
LAST_RESULTS = {}


def build_kernel_a(T=T_FULL):
    """Per core: 2 heads. Computes A.T = softmax(QK^T/sqrt(d)) @ V, transposed
    ([128 = 2*64 head dims, T] bf16) and normalized."""
    nc = bacc.Bacc("TRN2", target_bir_lowering=False, debug=False)
    KD = D // 128          # 8 contraction tiles over D
    NT = T // 128          # token tiles of 128
    NQ = T // 512          # query chunks of 512

    xt_d = nc.dram_tensor("xt", [128, KD, T], BF16, kind="ExternalInput")
    id_d = nc.dram_tensor("ident", [128, 128], BF16, kind="ExternalInput")
    tm_d = nc.dram_tensor("trimask", [128, 128], BF16, kind="ExternalInput")
    wq_d = nc.dram_tensor("wq_t", [128, KD, 128], BF16, kind="ExternalInput")
    wk_d = nc.dram_tensor("wk_t", [128, KD, 128], BF16, kind="ExternalInput")
    wv_d = nc.dram_tensor("wv_t", [128, KD, 128], BF16, kind="ExternalInput")
    bq_d = nc.dram_tensor("bq", [128, 1], F32, kind="ExternalInput")
    bk_d = nc.dram_tensor("bk", [128, 1], F32, kind="ExternalInput")
    bv_d = nc.dram_tensor("bv", [128, 1], F32, kind="ExternalInput")
    at_d = nc.dram_tensor("at_out", [128, T], BF16, kind="ExternalOutput")

    with tile.TileContext(nc) as tc, ExitStack() as ctx:
        const = ctx.enter_context(tc.tile_pool(name="const", bufs=1))
        persist = ctx.enter_context(tc.tile_pool(name="persist", bufs=1))
        vtp = ctx.enter_context(tc.tile_pool(name="vtp", bufs=2))
        e_pool = ctx.enter_context(tc.tile_pool(name="e_pool", bufs=4))
        rb_pool = ctx.enter_context(tc.tile_pool(name="rb_pool", bufs=2))
        s_ps = ctx.enter_context(tc.tile_pool(name="s_ps", bufs=2, space="PSUM"))
        pv_ps = ctx.enter_context(tc.tile_pool(name="pv_ps", bufs=1, space="PSUM"))
        mm_ps = ctx.enter_context(tc.tile_pool(name="mm_ps", bufs=1, space="PSUM"))
        tr_ps = ctx.enter_context(tc.tile_pool(name="tr_ps", bufs=1, space="PSUM"))

        ident = const.tile([128, 128], BF16)
        nc.sync.dma_start(ident[:], id_d.ap())
        trimask = const.tile([128, 128], BF16)
        nc.sync.dma_start(trimask[:], tm_d.ap())
        wq_sb = const.tile([128, KD, 128], BF16, tag="wq")
        wk_sb = const.tile([128, KD, 128], BF16, tag="wk")
        wv_sb = const.tile([128, KD, 128], BF16, tag="wv")
        nc.sync.dma_start(wq_sb[:], wq_d.ap())
        nc.sync.dma_start(wk_sb[:], wk_d.ap())
        nc.sync.dma_start(wv_sb[:], wv_d.ap())
        bq_sb = const.tile([128, 1], F32, tag="bq")
        bk_sb = const.tile([128, 1], F32, tag="bk")
        bv_sb = const.tile([128, 1], F32, tag="bv")
        nc.sync.dma_start(bq_sb[:], bq_d.ap())
        nc.sync.dma_start(bk_sb[:], bk_d.ap())
        nc.sync.dma_start(bv_sb[:], bv_d.ap())

        xt_sb = persist.tile([128, KD, T], BF16, tag="xt")
        # V natural layout [k-token part, kt, 130]: per head 64 V cols + ones
        v_sb = persist.tile([128, NT, 130], BF16, tag="v")
        nc.gpsimd.memset(v_sb[:, :, 64:65], 1.0)
        nc.gpsimd.memset(v_sb[:, :, 129:130], 1.0)
        qt_sb = persist.tile([128, T], BF16, tag="qt")
        kt_sb = persist.tile([128, T], BF16, tag="kt")
        at_sb = persist.tile([128, T], BF16, tag="at")

        for qc in range(NQ):
            c_sl = slice(qc * 512, (qc + 1) * 512)
            # ---- QKV projection for chunk qc ----
            nc.sync.dma_start(xt_sb[:, :, c_sl], xt_d.ap()[:, :, c_sl])
            for nm, w_sb, b_sb, o_sb in (("q", wq_sb, bq_sb, qt_sb),
                                         ("k", wk_sb, bk_sb, kt_sb)):
                pps = mm_ps.tile([128, 512], F32, tag="mm", name=f"pps_{nm}_{qc}")
                for kt in range(KD):
                    nc.tensor.matmul(pps[:], w_sb[:, kt, :], xt_sb[:, kt, c_sl],
                                     start=(kt == 0), stop=(kt == KD - 1))
                nc.scalar.activation(out=o_sb[:, c_sl], in_=pps[:], func=IDENT_FN,
                                     bias=b_sb[:], scale=1.0)
            vps = mm_ps.tile([128, 512], F32, tag="mm", name=f"vps_{qc}")
            for kt in range(KD):
                nc.tensor.matmul(vps[:], wv_sb[:, kt, :], xt_sb[:, kt, c_sl],
                                 start=(kt == 0), stop=(kt == KD - 1))
            vt_c = vtp.tile([128, 512], BF16, tag="vt", name=f"vt_{qc}")
            nc.scalar.activation(out=vt_c[:], in_=vps[:], func=IDENT_FN,
                                 bias=bv_sb[:], scale=1.0)
            tpv = tr_ps.tile([128, 4, 128], BF16, tag="tr", name=f"tpv_{qc}")
            for q in range(4):
                nc.tensor.transpose(tpv[:, q, :], vt_c[:, q * 128:(q + 1) * 128],
                                    ident[:])
            nc.vector.tensor_copy(v_sb[:, qc * 4:(qc + 1) * 4, 0:64],
                                  tpv[:, :, 0:64])
            nc.vector.tensor_copy(v_sb[:, qc * 4:(qc + 1) * 4, 65:129],
                                  tpv[:, :, 64:128])

            # ---- attention for chunk qc ----
            nkt = 4 * (qc + 1)
            q_sl = c_sl
            pv = [pv_ps.tile([65, 512], F32, tag=f"pv{h}", name=f"pv{h}_{qc}")
                  for h in (0, 1)]

            def emit_pv(kt, esb):
                for h in (0, 1):
                    nc.tensor.matmul(pv[h][:, :],
                                     v_sb[:, kt, 65 * h:65 * h + 65],
                                     esb[:, 512 * h:512 * h + 512],
                                     start=(kt == 0), stop=(kt == nkt - 1),
                                     skip_group_check=True)

            prev = None
            for kt in range(nkt):
                sp = s_ps.tile([128, 1024], F32, tag="s", name=f"s_{qc}_{kt}")
                for h in (0, 1):
                    h_sl = slice(64 * h, 64 * h + 64)
                    nc.tensor.matmul(sp[:, 512 * h:512 * h + 512],
                                     kt_sb[h_sl, kt * 128:(kt + 1) * 128],
                                     qt_sb[h_sl, q_sl],
                                     start=True, stop=True)
                esb = e_pool.tile([128, 1024], BF16, tag="e", name=f"e_{qc}_{kt}")
                if kt % 2 == 0:
                    nc.scalar.activation(out=esb[:], in_=sp[:], func=EXP)
                else:
                    nc.vector.tensor_scalar(out=esb[:].bitcast(I16), in0=sp[:],
                                            scalar1=SCH_A, scalar2=SCH_B,
                                            op0=mybir.AluOpType.mult,
                                            op1=mybir.AluOpType.add)
                if kt >= nkt - 4:
                    o = kt * 128 - qc * 512
                    for h in (0, 1):
                        if o > 0:
                            nc.gpsimd.memset(esb[:, 512 * h:512 * h + o], 0.0)
                        nc.gpsimd.tensor_mul(esb[:, 512 * h + o:512 * h + o + 128],
                                             esb[:, 512 * h + o:512 * h + o + 128],
                                             trimask[:])
                if prev is not None:
                    emit_pv(kt - 1, prev)
                prev = esb
            emit_pv(nkt - 1, prev)

            for h in (0, 1):
                r1 = rb_pool.tile([1, 512], F32, tag="r1", name=f"r1{h}_{qc}")
                nc.scalar.copy(r1[:], pv[h][64:65, :])
                rb = rb_pool.tile([128, 512], F32, tag="rb", name=f"rb{h}_{qc}")
                nc.gpsimd.partition_broadcast(rb[:], r1[:], channels=128)
                nc.vector.reciprocal_approx_fast(out=rb[:], in_=rb[:])
                nc.vector.tensor_mul(at_sb[64 * h:64 * h + 64, q_sl],
                                     pv[h][0:64, :], rb[64 * h:64 * h + 64, :])
            nc.sync.dma_start(at_d.ap()[:, q_sl], at_sb[:, q_sl])

    nc.compile()
    return nc


def build_kernel_b(T=T_FULL):
    """Per core: rows slice of T/8 tokens: out-proj (bf16) + residual (+bout
    folded on host into xb) + LayerNorm*gamma+beta. gamma/beta pre-broadcast
    on host; gpsimd untouched (its teardown drain costs ~50us)."""
    nc = bacc.Bacc("TRN2", target_bir_lowering=False, debug=False)
    Tc = T // NCORES
    KD = D // 128

    at_d = nc.dram_tensor("at", [128, KD, Tc], BF16, kind="ExternalInput")
    wo_d = nc.dram_tensor("wout_t", [128, KD, D], BF16, kind="ExternalInput")
    xb_d = nc.dram_tensor("xb", [Tc, D], F32, kind="ExternalInput")
    g_d = nc.dram_tensor("gamma", [128, D], F32, kind="ExternalInput")
    be_d = nc.dram_tensor("beta", [128, D], F32, kind="ExternalInput")
    y_d = nc.dram_tensor("y", [Tc, D], F32, kind="ExternalOutput")

    with tile.TileContext(nc) as tc, ExitStack() as ctx:
        const = ctx.enter_context(tc.tile_pool(name="const", bufs=1))
        work = ctx.enter_context(tc.tile_pool(name="work", bufs=2))
        stats = ctx.enter_context(tc.tile_pool(name="stats", bufs=4))
        ps = ctx.enter_context(tc.tile_pool(name="ps", bufs=4, space="PSUM"))

        at_sb = const.tile([128, KD, Tc], BF16, tag="at")
        nc.sync.dma_start(at_sb[:], at_d.ap())
        wo_sb = const.tile([128, KD, D], BF16, tag="wo")
        nc.sync.dma_start(wo_sb[:], wo_d.ap())
        gam_b = const.tile([128, D], F32, tag="gam")
        bet_b = const.tile([128, D], F32, tag="bet")
        nc.sync.dma_start(gam_b[:], g_d.ap())
        nc.sync.dma_start(bet_b[:], be_d.ap())
        eps_sb = const.tile([128, 1], F32, tag="eps")
        nc.vector.memset(eps_sb[:], LN_EPS)

        for tt in range(Tc // 128):
            t_sl = slice(tt * 128, (tt + 1) * 128)
            xb_t = work.tile([128, D], F32, tag="xb")
            nc.sync.dma_start(xb_t[:], xb_d.ap()[t_sl, :])
            y_t = work.tile([128, D], F32, tag="y")
            for j in (0, 1):
                pp = ps.tile([128, 512], F32, tag="pp")
                for kt in range(KD):
                    nc.tensor.matmul(pp[:], at_sb[:, kt, t_sl],
                                     wo_sb[:, kt, j * 512:(j + 1) * 512],
                                     start=(kt == 0), stop=(kt == KD - 1))
                nc.vector.tensor_add(y_t[:, j * 512:(j + 1) * 512], pp[:],
                                     xb_t[:, j * 512:(j + 1) * 512])
            st = stats.tile([128, 2, 6], F32, tag="st")
            nc.vector.bn_stats(st[:, 0, :], y_t[:, 0:512])
            nc.vector.bn_stats(st[:, 1, :], y_t[:, 512:1024])
            mv = stats.tile([128, 2], F32, tag="mv")
            nc.vector.bn_aggr(mv[:], st[:])
            sq = stats.tile([128, 1], F32, tag="sq")
            nc.scalar.activation(out=sq[:], in_=mv[:, 1:2], func=SQRT,
                                 bias=eps_sb[:], scale=1.0)
            rstd = stats.tile([128, 1], F32, tag="rstd")
            nc.vector.reciprocal(rstd[:], sq[:])
            nc.vector.tensor_scalar(out=y_t[:], in0=y_t[:], scalar1=mv[:, 0:1],
                                    scalar2=rstd[:], op0=mybir.AluOpType.subtract,
                                    op1=mybir.AluOpType.mult)
            nc.vector.tensor_mul(y_t[:], y_t[:], gam_b[:])
            nc.vector.tensor_add(y_t[:], y_t[:], bet_b[:])
            nc.sync.dma_start(y_d.ap()[t_sl, :], y_t[:])

    nc.compile()
    return nc


def _get_kernels(T=T_FULL):
    if T not in _CACHE:
        _CACHE[T] = (build_kernel_a(T), build_kernel_b(T))
    return _CACHE[T]


def _tile_kd(a):
    """[D, M] -> [128, D//128, M] with row = kt*128 + p."""
    Dd, M = a.shape
    return np.ascontiguousarray(a.reshape(Dd // 128, 128, M).transpose(1, 0, 2))


def kernel(x, Wqkv, bqkv, Wout, bout, gamma, beta):
    x = np.asarray(x, dtype=np.float32)
    Wqkv = np.asarray(Wqkv, dtype=np.float32)
    bqkv = np.asarray(bqkv, dtype=np.float32)
    Wout = np.asarray(Wout, dtype=np.float32)
    bout = np.asarray(bout, dtype=np.float32)
    gamma = np.asarray(gamma, dtype=np.float32)
    beta = np.asarray(beta, dtype=np.float32)

    B, T, D_ = x.shape
    assert B == 1 and D_ == D
    d = D // HEADS
    scale = d ** -0.5
    x2d = np.ascontiguousarray(x[0])
    ident = np.eye(128, dtype=np.float32).astype(BF)
    tri = np.triu(np.ones((128, 128), np.float32)).astype(BF)

    nc_a, nc_b = _get_kernels(T)

    xt = _tile_kd(x2d.T.astype(BF))            # [128, 8, T] bf16
    in_maps_a = []
    for c in range(NCORES):
        r = slice(c * 128, (c + 1) * 128)
        wq = Wqkv[0 * D:1 * D][r]
        wk = Wqkv[1 * D:2 * D][r] * scale
        wv = Wqkv[2 * D:3 * D][r]
        in_maps_a.append({
            "xt": xt,
            "ident": ident,
            "trimask": tri,
            "wq_t": _tile_kd(wq.T.astype(BF)),
            "wk_t": _tile_kd(wk.T.astype(BF)),
            "wv_t": _tile_kd(wv.T.astype(BF)),
            "bq": np.ascontiguousarray(bqkv[0 * D:1 * D][r].reshape(128, 1)),
            "bk": np.ascontiguousarray((bqkv[1 * D:2 * D][r] * scale).reshape(128, 1)),
            "bv": np.ascontiguousarray(bqkv[2 * D:3 * D][r].reshape(128, 1)),
        })
    res_a = run_bass_kernel_spmd(nc_a, in_maps_a, core_ids=list(range(NCORES)))
    LAST_RESULTS["a"] = res_a
    at_full = np.concatenate([np.asarray(res_a.results[c]["at_out"])
                              for c in range(NCORES)], axis=0)  # [D, T] bf16

    Tc = T // NCORES
    wo_tiled = _tile_kd(Wout.T.astype(BF))     # [128, 8, D] bf16
    gam_b = np.ascontiguousarray(np.broadcast_to(gamma.reshape(1, D), (128, D)))
    bet_b = np.ascontiguousarray(np.broadcast_to(beta.reshape(1, D), (128, D)))
    in_maps_b = []
    for c in range(NCORES):
        t_sl = slice(c * Tc, (c + 1) * Tc)
        at_c = at_full[:, t_sl]                # [D, Tc] bf16
        in_maps_b.append({
            "at": _tile_kd(at_c),
            "wout_t": wo_tiled,
            "xb": np.ascontiguousarray(x2d[t_sl] + bout[None, :]),
            "gamma": gam_b,
            "beta": bet_b,
        })
    res_b = run_bass_kernel_spmd(nc_b, in_maps_b, core_ids=list(range(NCORES)))
    LAST_RESULTS["b"] = res_b
    y = np.concatenate([res_b.results[c]["y"] for c in range(NCORES)], axis=0)
    return y.reshape(1, T, D).astype(np.float32)
